# revision 41
# baseline (speedup 1.0000x reference)
"""MoE layer (GShard top-2 routing + per-expert FFN) on 8 Trainium2 NeuronCores.

Strategy (expert parallelism):
  - Router matmul (fp32, exact) is token-sharded: each core computes logits for
    its 1024-token shard, then an AllGather shares per-token routing scalars
    (idx1, idx2, g1, g2) with all cores.
  - Every core replicates the (cheap) global slot-assignment math: per-expert
    inclusive scans along the free dim + a triangular-matmul partition prefix
    give each token its capacity slot exactly as the reference's cumsum does.
  - Each core owns ONE expert. The slot->token map is built with local_scatter
    (per-partition scatter of token ids by slot), merged across partitions with
    a ones-matmul, and read out column-major via a diagonal extraction.
  - Dispatch: 16 indirect row gathers from x (bf16) + PE transposes give the
    [d, slot] layout; FFN in bf16 with fp32 accumulation:
    hT = gelu(w_gate^T @ dispT), eo = hT^T @ w_down (row-major out).
  - Combine via ReduceScatter: each expert core multiplies its eo rows by the
    per-slot combine gate (fused into the PSUM->SBUF copy), scatters them by
    token id into a zeroed [T, D] bf16 buffer (sentinel slots skipped via the
    indirect-DMA bounds check), and one ReduceScatter sums the 8 buffers and
    hands each core its 1024-token shard, which is cast to fp32 as y.
"""

import sys

if "/opt/trn_rl_repo" not in sys.path:
    sys.path.insert(0, "/opt/trn_rl_repo")

import numpy as np
import ml_dtypes

import concourse.bacc as bacc
import concourse.mybir as mybir
import concourse.tile as tile
from concourse import bass
from concourse.bass_utils import run_bass_kernel_spmd

BF16 = mybir.dt.bfloat16
F32 = mybir.dt.float32
I16 = mybir.dt.int16
I32 = mybir.dt.int32
AF = mybir.ActivationFunctionType
OP = mybir.AluOpType

B, S, D, E, F = 4, 2048, 1024, 8, 4096
T = B * S            # 8192 tokens
C = 2 * T // E       # 2048 capacity
NC = 8               # cores
SH = T // NC         # 1024 tokens per shard
CBLK = 512           # FFN slot-block
NCB = C // CBLK      # 4 blocks

LAST_RESULT = None   # BassKernelResults of the most recent run (for profiling)


def _build_program():
    nc = bacc.Bacc("TRN2", target_bir_lowering=False, debug=False, num_devices=NC)

    # ---- per-core external inputs ----
    xT_sh = nc.dram_tensor("xT_sh", [D, SH], F32, kind="ExternalInput").ap()
    xb = nc.dram_tensor("xb", [T + 1, D], BF16, kind="ExternalInput").ap()
    wg_d = nc.dram_tensor("wg", [D, E], F32, kind="ExternalInput").ap()
    wgt_d = nc.dram_tensor("wgt", [D, F], BF16, kind="ExternalInput").ap()
    wdn_d = nc.dram_tensor("wdn", [F, D], BF16, kind="ExternalInput").ap()
    cid_d = nc.dram_tensor("cid", [128, 1], F32, kind="ExternalInput").ap()
    # host-generated constants (gpsimd iota/affine_select aren't available)
    ident_d = nc.dram_tensor("ident", [128, 128], F32, kind="ExternalInput").ap()
    slmat_d = nc.dram_tensor("slmat", [128, 128], F32, kind="ExternalInput").ap()
    tidx_d = nc.dram_tensor("tidx", [128, 64], F32, kind="ExternalInput").ap()
    eidx_d = nc.dram_tensor("eidx", [128, E], F32, kind="ExternalInput").ap()
    y_d = nc.dram_tensor("y", [SH, D], F32, kind="ExternalOutput").ap()

    # ---- internal DRAM ----
    pay_in = nc.dram_tensor("pay_in", [4 * SH], F32).ap()
    pay_all = nc.dram_tensor("pay_all", [NC * 4 * SH], F32, addr_space="Shared").ap()
    rt_d = nc.dram_tensor("rt_d", [T + 1, 4], F32).ap()       # per-token (idx1, g1k, g2k, 0)
    contrib = nc.dram_tensor("contrib", [T, D], BF16).ap()    # this expert's combine contribution
    rs_out = nc.dram_tensor("rs_out", [SH, D], BF16).ap()     # ReduceScatter result (token shard)

    with tile.TileContext(nc) as tc:
        with (
            tc.tile_pool(name="persist", bufs=1) as pp,
            tc.tile_pool(name="psum_s", bufs=2, space="PSUM") as pss,
        ):
            ident = pp.tile([128, 128], F32)
            nc.sync.dma_start(ident[:], ident_d[:])
            ident_bf = pp.tile([128, 128], BF16)
            nc.vector.tensor_copy(ident_bf[:], ident[:])
            cid = pp.tile([128, 1], F32)
            nc.sync.dma_start(cid[:], cid_d[:])
            zeros64 = pp.tile([128, 64], F32)
            nc.vector.memset(zeros64[:], 0.0)
            ones128 = pp.tile([128, 128], F32)
            nc.vector.memset(ones128[:], 1.0)

            # resident expert weights (bf16); DMAs issued after the router
            # section so the router's xT load goes out first
            wgt_sb = pp.tile([128, D // 128, F], BF16)
            wdn_sb = pp.tile([128, F // 128, D], BF16)

            # persistent routing products
            tokc = pp.tile([128, C // 128], I32)    # dispatch: slot->token, col-major
            zbf = pp.tile([128, 1024], BF16)        # zero source for contrib fill

            # =============== ROUTER (token shard, fp32) ===============
            with (
                tc.tile_pool(name="route", bufs=1) as pr,
                tc.tile_pool(name="psum_mg", bufs=1, space="PSUM") as psd,
            ):
                # DMA issue order matters (transfers serialize): router inputs
                # first, then FFN weights, then the contrib zero-fill, which
                # only needs to land before the first FFN scatter
                eidx = pr.tile([128, E], F32)
                nc.sync.dma_start(eidx[:], eidx_d[:])
                sl = pr.tile([128, 128], F32)
                nc.sync.dma_start(sl[:], slmat_d[:])
                tif = pr.tile([128, 64], F32)
                nc.sync.dma_start(tif[:], tidx_d[:])

                wg_sb = pr.tile([128, D // 128, E], F32)
                nc.sync.dma_start(wg_sb[:], wg_d.rearrange("(o q) e -> q o e", q=128))
                xT_a = pr.tile([128, D // 128, SH // 2], F32)
                xT_b = pr.tile([128, D // 128, SH // 2], F32)
                xT_v = xT_sh.rearrange("(o q) t -> q o t", q=128)
                nc.sync.dma_start(xT_a[:], xT_v[:, :, : SH // 2])
                nc.sync.dma_start(xT_b[:], xT_v[:, :, SH // 2 :])

                lg = pr.tile([128, 8, E], F32)  # logits, token pos j = 128*tt + p
                for tt in range(8):
                    half = xT_a if tt < 4 else xT_b
                    toff = 128 * (tt % 4)
                    ps = pss.tile([128, E], F32, space="PSUM", tag="ps_small")
                    for kd in range(8):
                        nc.tensor.matmul(
                            ps[:],
                            lhsT=half[:, kd, toff : toff + 128],
                            rhs=wg_sb[:, kd, :],
                            start=(kd == 0),
                            stop=(kd == 7),
                        )
                    nc.vector.tensor_copy(lg[:, tt, :], ps[:])

                def emax(src, width, tag):
                    red = pr.tile([128, 8, 1], F32, tag=f"emax{tag}")
                    nc.vector.reduce_max(red[:], src[:], axis=mybir.AxisListType.X)
                    return red  # [128, 8, 1]

                m1x = emax(lg, E, "m1")
                is1 = pr.tile([128, 8, E], F32)
                nc.vector.tensor_tensor(out=is1[:], in0=lg[:], in1=m1x[:].to_broadcast([128, 8, E]), op=OP.is_equal)
                l2 = pr.tile([128, 8, E], F32)
                nc.vector.scalar_tensor_tensor(
                    out=l2[:], in0=is1[:], scalar=-1e30, in1=lg[:], op0=OP.mult, op1=OP.add,
                )
                m2x = emax(l2, E, "m2")
                is2 = pr.tile([128, 8, E], F32)
                nc.vector.tensor_tensor(out=is2[:], in0=l2[:], in1=m2x[:].to_broadcast([128, 8, E]), op=OP.is_equal)

                dm = pr.tile([128, 8, 1], F32)
                nc.vector.tensor_tensor(out=dm[:], in0=m2x[:], in1=m1x[:], op=OP.subtract)
                e2 = pr.tile([128, 8, 1], F32)
                nc.scalar.activation(e2[:], dm[:], AF.Exp)
                den = pr.tile([128, 8, 1], F32)
                nc.vector.tensor_scalar_add(den[:], e2[:], 1.0)
                g1 = pr.tile([128, 8, 1], F32)
                nc.vector.reciprocal(g1[:], den[:])
                g2 = pr.tile([128, 8, 1], F32)
                nc.vector.tensor_tensor(out=g2[:], in0=e2[:], in1=g1[:], op=OP.mult)

                def argmax_num(mask, tag):
                    t1 = pr.tile([128, 8, E], F32, tag=f"am_t1{tag}")
                    nc.vector.tensor_tensor(
                        out=t1[:], in0=mask[:], in1=eidx[:, None, :].to_broadcast([128, 8, E]), op=OP.mult,
                    )
                    red = pr.tile([128, 8, 1], F32, tag=f"am_r{tag}")
                    nc.vector.reduce_sum(red[:], t1[:], axis=mybir.AxisListType.X)
                    return red  # [128, 8, 1]

                i1f = argmax_num(is1, "a")
                i2f = argmax_num(is2, "b")

                pk = pr.tile([128, 4, 8], F32)
                nc.vector.tensor_copy(pk[:, 0, :], i1f[:, :, 0])
                nc.vector.tensor_copy(pk[:, 1, :], i2f[:, :, 0])
                nc.vector.tensor_copy(pk[:, 2, :], g1[:, :, 0])
                nc.vector.tensor_copy(pk[:, 3, :], g2[:, :, 0])
                nc.sync.dma_start(pay_in.rearrange("(a p tt) -> p a tt", a=4, p=128), pk[:])

                nc.gpsimd.collective_compute(
                    "AllGather", OP.bypass,
                    replica_groups=[list(range(NC))],
                    ins=[pay_in[:].opt()], outs=[pay_all[:].opt()],
                )

                # reread all 4 arrays into global routing layout [128, 64] (t = 64p + i)
                rt = pr.tile([128, 4, 64], F32)
                pay_view = pay_all.rearrange("(r a p16 i) -> r p16 a i", r=NC, a=4, p16=16)
                for r in range(NC):
                    nc.sync.dma_start(rt[16 * r : 16 * r + 16, :, :], pay_view[r])
                i1r, i2r = rt[:, 0, :], rt[:, 1, :]
                g1r, g2r = rt[:, 2, :], rt[:, 3, :]

                # gate_proj weights issue on the SP queue BEHIND the payload
                # and the AllGather re-reads (the SP sequencer stalls on the
                # collective there), so this bulk traffic cannot delay the
                # router's critical path; it drains during slot assignment.
                # (down_proj loads + contrib zero-fill are issued inside the
                # FFN loop so their shared-semaphore thresholds cannot stall
                # the first mm1.)
                wgt_v = wgt_d.rearrange("(o q) f -> q o f", q=128)
                for o in range(D // 128):
                    nc.sync.dma_start(wgt_sb[:, o, :], wgt_v[:, o, :])

                # =============== SLOT ASSIGNMENT (replicated) ===============
                # masks for all 8 experts in one broadcast is_equal; per-expert
                # inclusive cumsum via one flat scan + segment-end correction
                m1 = pr.tile([128, E, 64], F32)
                m2 = pr.tile([128, E, 64], F32)
                eb = eidx[:, :, None].to_broadcast([128, E, 64])
                nc.vector.tensor_tensor(out=m1[:], in0=rt[:, 0:1, :].to_broadcast([128, E, 64]), in1=eb, op=OP.is_equal)
                nc.vector.tensor_tensor(out=m2[:], in0=rt[:, 1:2, :].to_broadcast([128, E, 64]), in1=eb, op=OP.is_equal)
                sc1 = pr.tile([128, E, 64], F32)
                sc2 = pr.tile([128, E, 64], F32)
                fl1 = pr.tile([128, E, 64], F32, tag="fl")
                zb = zeros64[:, 0:1].to_broadcast([128, E * 64])
                nc.vector.tensor_tensor_scan(
                    fl1[:].rearrange("p e i -> p (e i)"), m1[:].rearrange("p e i -> p (e i)"),
                    zb, 0.0, op0=OP.add, op1=OP.add,
                )
                nc.vector.tensor_copy(sc1[:, 0, :], fl1[:, 0, :])
                nc.vector.tensor_tensor(
                    out=sc1[:, 1:, :], in0=fl1[:, 1:, :],
                    in1=fl1[:, : E - 1, 63:64].to_broadcast([128, E - 1, 64]), op=OP.subtract,
                )
                fl2 = pr.tile([128, E, 64], F32, tag="fl")
                nc.vector.tensor_tensor_scan(
                    fl2[:].rearrange("p e i -> p (e i)"), m2[:].rearrange("p e i -> p (e i)"),
                    zb, 0.0, op0=OP.add, op1=OP.add,
                )
                nc.vector.tensor_copy(sc2[:, 0, :], fl2[:, 0, :])
                nc.vector.tensor_tensor(
                    out=sc2[:, 1:, :], in0=fl2[:, 1:, :],
                    in1=fl2[:, : E - 1, 63:64].to_broadcast([128, E - 1, 64]), op=OP.subtract,
                )
                tot1 = pr.tile([128, E], F32)
                tot2 = pr.tile([128, E], F32)
                nc.vector.tensor_copy(tot1[:], sc1[:, :, 63])
                nc.vector.tensor_copy(tot2[:], sc2[:, :, 63])

                of1_ps = pss.tile([128, E], F32, space="PSUM", tag="ps_small")
                nc.tensor.matmul(of1_ps[:], lhsT=sl[:], rhs=tot1[:], start=True, stop=True)
                of1 = pr.tile([128, E], F32)
                nc.vector.tensor_scalar_add(of1[:], of1_ps[:], -1.0)
                of2_ps = pss.tile([128, E], F32, space="PSUM", tag="ps_small")
                nc.tensor.matmul(of2_ps[:], lhsT=sl[:], rhs=tot2[:], start=True, stop=False)
                nc.tensor.matmul(of2_ps[:], lhsT=ones128[:], rhs=tot1[:], start=False, stop=True)
                of2 = pr.tile([128, E], F32)
                nc.vector.tensor_scalar_add(of2[:], of2_ps[:], -1.0)

                def loc_s(sc, m, of, tag, eng=None):
                    eng = eng or nc.vector
                    tmp = pr.tile([128, E, 64], F32, tag=f"loc_tmp{tag}")
                    eng.tensor_tensor(
                        out=tmp[:], in0=sc[:],
                        in1=of[:, :, None].to_broadcast([128, E, 64]), op=OP.add,
                    )
                    eng.tensor_tensor(out=tmp[:], in0=tmp[:], in1=m[:], op=OP.mult)
                    red = pr.tile([128, 64, 1], F32, tag=f"loc_r{tag}")
                    nc.vector.reduce_sum(
                        red[:], tmp[:].rearrange("p e i -> p i e"), axis=mybir.AxisListType.X,
                    )
                    return red  # [128, 64, 1]

                l1s = loc_s(sc1, m1, of1, "a")[:, :, 0]
                l2s = loc_s(sc2, m2, of2, "b")[:, :, 0]

                def keep_g(ls, gr, tag, eng=None):
                    eng = eng or nc.vector
                    kp = pr.tile([128, 64], F32, tag=f"kp{tag}")
                    eng.tensor_scalar(out=kp[:], in0=ls, scalar1=float(C), scalar2=None, op0=OP.is_lt)
                    gk = pr.tile([128, 64], F32, tag=f"gk{tag}")
                    eng.tensor_tensor(out=gk[:], in0=gr, in1=kp[:], op=OP.mult)
                    return gk, kp

                g1k, kp1 = keep_g(l1s, g1r, "a")
                g2k, kp2 = keep_g(l2s, g2r, "b")

                # per-token routing table -> DRAM: row t = (idx1, g1k, g2k, 0)
                rtt = pr.tile([128, 64, 4], F32)
                nc.vector.tensor_copy(rtt[:, :, 0], i1r)
                nc.vector.tensor_copy(rtt[:, :, 1], g1k[:])
                nc.vector.tensor_copy(rtt[:, :, 2], g2k[:])
                nc.vector.memset(rtt[:, :, 3], 0.0)
                nc.sync.dma_start(rt_d[0:T, :].rearrange("(p i) c -> p i c", p=128), rtt[:])
                nc.sync.dma_start(rt_d[T : T + 1, :], zeros64[0:1, 0:4])

                # ====== SLOT -> TOKEN MAP (local_scatter + merge + diagonal) ======
                tp1 = pr.tile([128, 64], F32)
                nc.vector.tensor_scalar_add(tp1[:], tif[:], 1.0)   # token id + 1

                def slot_halves(ls, ir, kp, tag, eng=None):
                    eng = eng or nc.vector
                    # sel = (expert == cid) && kept; slot+1 where selected else 0
                    isc = pr.tile([128, 64], F32, tag=f"isc{tag}")
                    eng.tensor_tensor(out=isc[:], in0=ir, in1=cid[:, 0:1].to_broadcast([128, 64]), op=OP.is_equal)
                    sel = pr.tile([128, 64], F32, tag=f"sel{tag}")
                    eng.tensor_tensor(out=sel[:], in0=isc[:], in1=kp[:], op=OP.mult)
                    sp1 = pr.tile([128, 64], F32, tag=f"sp1{tag}")  # sel ? slot+1 : 0
                    eng.tensor_scalar_add(sp1[:], ls, 1.0)
                    eng.tensor_tensor(out=sp1[:], in0=sp1[:], in1=sel[:], op=OP.mult)
                    # lo half: slot in [0, 1024): idx = slot, else -1
                    mlo = pr.tile([128, 64], F32, tag=f"mlo{tag}")
                    eng.tensor_scalar(out=mlo[:], in0=sp1[:], scalar1=1024.0, scalar2=None, op0=OP.is_le)
                    eng.tensor_tensor(out=mlo[:], in0=mlo[:], in1=sel[:], op=OP.mult)
                    ilo = pr.tile([128, 64], F32, tag=f"ilo{tag}")
                    eng.tensor_tensor(out=ilo[:], in0=mlo[:], in1=sp1[:], op=OP.mult)
                    eng.tensor_scalar_add(ilo[:], ilo[:], -1.0)
                    # hi half: slot in [1024, 2048): idx = slot - 1024, else -1
                    mhi = pr.tile([128, 64], F32, tag=f"mhi{tag}")
                    eng.tensor_scalar(out=mhi[:], in0=sp1[:], scalar1=1024.0, scalar2=None, op0=OP.is_gt)
                    ihi = pr.tile([128, 64], F32, tag=f"ihi{tag}")
                    eng.tensor_scalar_add(ihi[:], sp1[:], -1024.0)
                    eng.tensor_tensor(out=ihi[:], in0=ihi[:], in1=mhi[:], op=OP.mult)
                    eng.tensor_scalar_add(ihi[:], ihi[:], -1.0)
                    return ilo, ihi

                i1lo, i1hi = slot_halves(l1s, i1r, kp1, "a")
                i2lo, i2hi = slot_halves(l2s, i2r, kp2, "b")

                data128 = pr.tile([128, 128], I16)
                nc.vector.tensor_copy(data128[:, :64], tp1[:])
                nc.vector.tensor_copy(data128[:, 64:], tp1[:])
                idxlo = pr.tile([128, 128], I16)
                nc.vector.tensor_copy(idxlo[:, :64], i1lo[:])
                nc.vector.tensor_copy(idxlo[:, 64:], i2lo[:])
                idxhi = pr.tile([128, 128], I16)
                nc.vector.tensor_copy(idxhi[:, :64], i1hi[:])
                nc.vector.tensor_copy(idxhi[:, 64:], i2hi[:])

                dst_lo = pr.tile([128, 1024], I16)
                nc.gpsimd.local_scatter(dst_lo[:], data128[:], idxlo[:], channels=128, num_elems=1024, num_idxs=128)
                dst_hi = pr.tile([128, 1024], I16)
                nc.gpsimd.local_scatter(dst_hi[:], data128[:], idxhi[:], channels=128, num_elems=1024, num_idxs=128)

                # merge across partitions with a ones-matmul, keep results in
                # PSUM (4 banks) and run the diagonal extraction straight off
                # them: tokraw[p, k] = merged-flat[128k + p]
                tokraw = pr.tile([128, C // 128], F32)
                scratch = pr.tile([128, 128], F32, tag="diag_scr")
                for half, dst in ((0, dst_lo), (1, dst_hi)):
                    dstf = pr.tile([128, 1024], F32, tag="dstf")
                    nc.vector.tensor_copy(dstf[:], dst[:])
                    for ch in range(2):
                        mg_ps = psd.tile([128, 512], F32, space="PSUM", tag=f"ps_mg{2 * half + ch}")
                        nc.tensor.matmul(mg_ps[:], lhsT=ones128[:], rhs=dstf[:, 512 * ch : 512 * (ch + 1)], start=True, stop=True)
                        for kk in range(4):
                            k = 4 * (2 * half + ch) + kk
                            nc.vector.scalar_tensor_tensor(
                                out=scratch[:], in0=mg_ps[:, 128 * kk : 128 * (kk + 1)], scalar=0.0,
                                in1=ident[:], op0=OP.add, op1=OP.mult,
                                accum_out=tokraw[:, k : k + 1],
                            )
                # sanitize: 0 -> T (zero row); v -> v-1
                iszero = pr.tile([128, C // 128], F32)
                nc.vector.tensor_scalar(out=iszero[:], in0=tokraw[:], scalar1=0.0, scalar2=None, op0=OP.is_equal)
                nc.vector.scalar_tensor_tensor(
                    out=tokraw[:], in0=iszero[:], scalar=float(T + 1), in1=tokraw[:], op0=OP.mult, op1=OP.add,
                )
                nc.vector.tensor_scalar_add(tokraw[:], tokraw[:], -1.0)
                nc.vector.tensor_copy(tokc[:], tokraw[:])

            # =============== EXPERT FFN (bf16) ===============
            with (
                tc.tile_pool(name="ffn", bufs=1) as pf,
                tc.tile_pool(name="ffn_db", bufs=2) as pfd,
                tc.tile_pool(name="psum_mm", bufs=2, space="PSUM") as psm,
            ):
                def fetch_block(cb):
                    # token rows + routing rows for block cb (prefetched one
                    # block ahead so the Pool-queue DMAs overlap compute)
                    drowb = pfd.tile([128, CBLK // 128, D], BF16, tag="drowb")
                    rtg = pfd.tile([128, CBLK // 128, 4], F32, tag="rtg")
                    for kt in range(CBLK // 128):
                        k = (CBLK // 128) * cb + kt
                        nc.gpsimd.indirect_dma_start(
                            out=drowb[:, kt, :], out_offset=None, in_=xb[:],
                            in_offset=bass.IndirectOffsetOnAxis(ap=tokc[:, k : k + 1], axis=0),
                        )
                        nc.gpsimd.indirect_dma_start(
                            out=rtg[:, kt, :], out_offset=None, in_=rt_d[:],
                            in_offset=bass.IndirectOffsetOnAxis(ap=tokc[:, k : k + 1], axis=0),
                        )
                    return drowb, rtg

                def transpose_block(drowb):
                    dispT = pfd.tile([128, D // 128, CBLK], BF16, tag="dispT")
                    for kt in range(CBLK // 128):
                        for dt in range(D // 128):
                            tr_ps = psm.tile([128, 128], BF16, space="PSUM", tag="ps_tr")
                            nc.tensor.transpose(tr_ps[:], drowb[:, kt, 128 * dt : 128 * (dt + 1)], ident_bf[:])
                            nc.vector.tensor_copy(dispT[:, dt, 128 * kt : 128 * (kt + 1)], tr_ps[:])
                    return dispT

                nxt = fetch_block(0)
                dispT = transpose_block(nxt[0])
                for cb in range(NCB):
                    drowb, rtg = nxt
                    if cb + 1 < NCB:
                        nxt = fetch_block(cb + 1)
                    # combine gate for this block's slots: idx1 match -> g1 else g2
                    gate_b = pfd.tile([128, CBLK // 128], F32, tag="gate_b")
                    isc = pfd.tile([128, CBLK // 128], F32, tag="isc")
                    nc.vector.tensor_tensor(
                        out=isc[:], in0=rtg[:, :, 0],
                        in1=cid[:, 0:1].to_broadcast([128, CBLK // 128]), op=OP.is_equal,
                    )
                    nc.vector.tensor_tensor(out=gate_b[:], in0=rtg[:, :, 1], in1=rtg[:, :, 2], op=OP.subtract)
                    nc.vector.tensor_tensor(out=gate_b[:], in0=gate_b[:], in1=isc[:], op=OP.mult)
                    nc.vector.tensor_tensor(out=gate_b[:], in0=gate_b[:], in1=rtg[:, :, 2], op=OP.add)

                    hT = pf.tile([128, F // 128, CBLK], BF16, tag="hT")
                    for ft in range(F // 128):
                        ps1 = psm.tile([128, CBLK], F32, space="PSUM", tag="ps1")
                        for kd in range(D // 128):
                            nc.tensor.matmul(
                                ps1[:],
                                lhsT=wgt_sb[:, kd, 128 * ft : 128 * ft + 128],
                                rhs=dispT[:, kd, :],
                                start=(kd == 0), stop=(kd == D // 128 - 1),
                            )
                        nc.scalar.activation(hT[:, ft, :], ps1[:], AF.Gelu)

                    if cb == 0:
                        # down_proj weights: linearly after mm1 so their DMAs
                        # never inflate mm1's wait thresholds; requested right
                        # behind the gate_proj chunks on the SP queue
                        wdn_v = wdn_d.rearrange("(o q) d -> q o d", q=128)
                        for o in range(F // 128 // 4):
                            nc.sync.dma_start(wdn_sb[:, 4 * o : 4 * o + 4, :], wdn_v[:, 4 * o : 4 * o + 4, :])
                        # contrib zero-fill (Activation queue), gated on the
                        # last wdn chunk so its traffic strictly follows the
                        # weights; linearly before the first scatter, which
                        # genuinely depends on it
                        nc.vector.tensor_scalar(
                            out=zbf[:], in0=wdn_sb[:, F // 128 - 1, :],
                            scalar1=0.0, scalar2=None, op0=OP.mult,
                        )
                        zc = contrib.rearrange("(o p) d -> p o d", p=128)
                        for q in range(T // 128):
                            nc.scalar.dma_start(zc[:, q, :], zbf[:])
                    # transposes for the next block run on the PE here, so
                    # dispT is ready the moment this block's mm2 retires
                    if cb + 1 < NCB:
                        dispT_next = transpose_block(nxt[0])

                    # mm2 with swapped operands: eo[c, d] = hT.T @ w_down -> row-major
                    # out, gated by the per-slot combine weight in the PSUM copy
                    eo_sb = pf.tile([128, CBLK // 128, D], BF16, tag="eo_sb")
                    for ct in range(CBLK // 128):
                        k = (CBLK // 128) * cb + ct
                        for dc in range(D // 512):
                            ps2 = psm.tile([128, 512], F32, space="PSUM", tag="ps2")
                            for ft in range(F // 128):
                                nc.tensor.matmul(
                                    ps2[:],
                                    lhsT=hT[:, ft, 128 * ct : 128 * ct + 128],
                                    rhs=wdn_sb[:, ft, 512 * dc : 512 * dc + 512],
                                    start=(ft == 0), stop=(ft == F // 128 - 1),
                                )
                            nc.vector.tensor_scalar_mul(
                                eo_sb[:, ct, 512 * dc : 512 * dc + 512], ps2[:],
                                gate_b[:, ct : ct + 1],
                            )
                        # scatter gated rows to their token position; sentinel
                        # (empty-slot) indices == T fail the bounds check and
                        # are silently dropped. The out AP is a 128-row window:
                        # indirect addressing only uses its base + row stride,
                        # and the sliced AP sizes the DMA as the 256 KiB it
                        # actually moves rather than the whole [T, D] tensor.
                        nc.gpsimd.indirect_dma_start(
                            out=contrib[0:128, :], out_offset=bass.IndirectOffsetOnAxis(ap=tokc[:, k : k + 1], axis=0),
                            in_=eo_sb[:, ct, :], in_offset=None,
                            bounds_check=T - 1, oob_is_err=False,
                        )
                    if cb + 1 < NCB:
                        dispT = dispT_next

                nc.gpsimd.collective_compute(
                    "ReduceScatter", OP.add,
                    replica_groups=[list(range(NC))],
                    ins=[contrib[:].opt()], outs=[rs_out[:].opt()],
                )

            # =============== OUTPUT CAST (token shard) ===============
            with tc.tile_pool(name="tail", bufs=2) as pt:
                NH = 2
                for h in range(NH):
                    nch = SH // 128 // NH
                    rsb = pt.tile([128, nch, D], BF16, tag="rsb")
                    nc.sync.dma_start(
                        rsb[:],
                        rs_out[SH // NH * h : SH // NH * (h + 1), :].rearrange("(c p) d -> p c d", p=128),
                    )
                    acc = pt.tile([128, nch, D], F32, tag="acc")
                    nc.vector.tensor_copy(acc[:], rsb[:])
                    nc.sync.dma_start(
                        y_d[SH // NH * h : SH // NH * (h + 1), :].rearrange("(c p) d -> p c d", p=128),
                        acc[:],
                    )

    nc.compile()
    return nc


_PROGRAM = None


def _get_program():
    global _PROGRAM
    if _PROGRAM is None:
        _PROGRAM = _build_program()
    return _PROGRAM


def host_constants():
    p = np.arange(128)
    return {
        "ident": np.eye(128, dtype=np.float32),
        "slmat": (np.arange(128)[None, :] > p[:, None]).astype(np.float32),
        "tidx": (64 * p[:, None] + np.arange(64)[None, :]).astype(np.float32),
        "eidx": np.tile(np.arange(E, dtype=np.float32), (128, 1)),
    }


def kernel(x, wg, w_gate, w_down, _trace=False):
    global LAST_RESULT
    x = np.asarray(x, np.float32)
    wg_np = np.asarray(wg, np.float32)
    w_gate_np = np.asarray(w_gate, np.float32)
    w_down_np = np.asarray(w_down, np.float32)

    tokens = x.reshape(T, D)
    xb = np.zeros((T + 1, D), ml_dtypes.bfloat16)
    xb[:T] = tokens.astype(ml_dtypes.bfloat16)

    # shard m holds tokens [SH*m, SH*(m+1)); its xT columns are permuted so that
    # matmul tile position j = 128*tt + p corresponds to local token 8*p + tt,
    # making the routing payload DMA contiguous.
    j = np.arange(SH)
    perm = 8 * (j % 128) + j // 128  # local token index at column position j
    consts = host_constants()

    in_maps = []
    for m in range(NC):
        shard = tokens[SH * m : SH * (m + 1)]
        xT_sh = np.ascontiguousarray(shard[perm].T)
        in_maps.append({
            "xT_sh": xT_sh,
            "xb": xb,
            "wg": wg_np,
            "wgt": np.ascontiguousarray(w_gate_np[m].astype(ml_dtypes.bfloat16)),
            "wdn": np.ascontiguousarray(w_down_np[m].astype(ml_dtypes.bfloat16)),
            "cid": np.full((128, 1), float(m), np.float32),
            **consts,
        })

    nc = _get_program()
    res = run_bass_kernel_spmd(nc, in_maps, core_ids=list(range(NC)), trace=_trace)
    LAST_RESULT = res
    out = np.concatenate([res.results[m]["y"] for m in range(NC)], axis=0)
    return out.reshape(B, S, D).astype(x.dtype)


def bench(x, wg, w_gate, w_down, iters=6):
    """Measure per-execution wall time with device-resident inputs.

    Returns (output, per_call_seconds_list, overhead_seconds_list) where
    overhead is measured with an empty jitted identity on the same mesh.
    """
    import time
    import jax
    from jax.sharding import Mesh, PartitionSpec, NamedSharding
    from jax.experimental.shard_map import shard_map
    import concourse.mybir as _mybir
    from concourse.bass2jax import _bass_exec_p, install_neuronx_cc_hook, partition_id_tensor

    install_neuronx_cc_hook()
    nc = _get_program()

    x = np.asarray(x, np.float32)
    tokens = x.reshape(T, D)
    xb = np.zeros((T + 1, D), ml_dtypes.bfloat16)
    xb[:T] = tokens.astype(ml_dtypes.bfloat16)
    j = np.arange(SH)
    perm = 8 * (j % 128) + j // 128
    consts = host_constants()
    w_gate_np = np.asarray(w_gate, np.float32)
    w_down_np = np.asarray(w_down, np.float32)
    in_maps = []
    for m in range(NC):
        shard = tokens[SH * m : SH * (m + 1)]
        in_maps.append({
            "xT_sh": np.ascontiguousarray(shard[perm].T),
            "xb": xb,
            "wg": np.asarray(wg, np.float32),
            "wgt": np.ascontiguousarray(w_gate_np[m].astype(ml_dtypes.bfloat16)),
            "wdn": np.ascontiguousarray(w_down_np[m].astype(ml_dtypes.bfloat16)),
            "cid": np.full((128, 1), float(m), np.float32),
            **consts,
        })

    in_names, out_names, out_avals, zero_outs = [], [], [], []
    for alloc in nc.m.functions[0].allocations:
        if not isinstance(alloc, _mybir.MemoryLocationSet):
            continue
        name = alloc.memorylocations[0].name
        if alloc.kind == "ExternalInput":
            if nc.partition_id_tensor is None or name != nc.partition_id_tensor.name:
                in_names.append(name)
        elif alloc.kind == "ExternalOutput":
            shape = tuple(alloc.tensor_shape)
            dtype = _mybir.dt.np(alloc.dtype)
            out_names.append(name)
            out_avals.append(jax.core.ShapedArray(shape, dtype))
            zero_outs.append(np.zeros(shape, dtype))
    n_params = len(in_names)
    all_in_names = in_names + out_names
    if nc.partition_id_tensor is not None:
        all_in_names = all_in_names + [nc.partition_id_tensor.name]

    def _body(*args):
        operands = list(args)
        if nc.partition_id_tensor is not None:
            operands.append(partition_id_tensor())
        outs = _bass_exec_p.bind(
            *operands,
            out_avals=tuple(out_avals),
            in_names=tuple(all_in_names),
            out_names=tuple(out_names),
            lowering_input_output_aliases=(),
            sim_require_finite=True,
            sim_require_nnan=True,
            nc=nc,
        )
        return tuple(outs)

    devices = jax.devices()[:NC]
    mesh = Mesh(np.asarray(devices), ("core",))
    nsh = NamedSharding(mesh, PartitionSpec("core"))
    n_outs = len(out_avals)
    donate = tuple(range(n_params, n_params + n_outs))
    sharded = jax.jit(
        shard_map(_body, mesh=mesh, in_specs=(PartitionSpec("core"),) * (n_params + n_outs),
                  out_specs=(PartitionSpec("core"),) * n_outs, check_rep=False),
        donate_argnums=donate, keep_unused=True,
    )

    concat_in = [
        jax.device_put(np.concatenate([np.asarray(in_maps[c][nm]) for c in range(NC)], axis=0), nsh)
        for nm in in_names
    ]
    zero_sets = [
        [jax.device_put(np.zeros((NC * z.shape[0], *z.shape[1:]), z.dtype), nsh) for z in zero_outs]
        for _ in range(iters + 1)
    ]

    out = sharded(*concat_in, *zero_sets[0])  # warmup + compile
    jax.block_until_ready(out)
    times = []
    for it in range(iters):
        t0 = time.perf_counter()
        out = sharded(*concat_in, *zero_sets[it + 1])
        jax.block_until_ready(out)
        times.append(time.perf_counter() - t0)

    outs = {
        nm: np.asarray(out[i]).reshape(NC, *out_avals[i].shape) for i, nm in enumerate(out_names)
    }
    y = np.concatenate([outs["y"][m] for m in range(NC)], axis=0).reshape(B, S, D).astype(x.dtype)
    return y, times



# revision 43
# speedup vs baseline: 1.0004x; 1.0004x over previous
"""MoE layer (GShard top-2 routing + per-expert FFN) on 8 Trainium2 NeuronCores.

Strategy (expert parallelism):
  - Router matmul (fp32, exact) is token-sharded: each core computes logits for
    its 1024-token shard, then an AllGather shares per-token routing scalars
    (idx1, idx2, g1, g2) with all cores.
  - Every core replicates the (cheap) global slot-assignment math: per-expert
    inclusive scans along the free dim + a triangular-matmul partition prefix
    give each token its capacity slot exactly as the reference's cumsum does.
  - Each core owns ONE expert. The slot->token map is built with local_scatter
    (per-partition scatter of token ids by slot), merged across partitions with
    a ones-matmul, and read out column-major via a diagonal extraction.
  - Dispatch: 16 indirect row gathers from x (bf16) + PE transposes give the
    [d, slot] layout; FFN in bf16 with fp32 accumulation:
    hT = gelu(w_gate^T @ dispT), eo = hT^T @ w_down (row-major out).
  - Combine via ReduceScatter: each expert core multiplies its eo rows by the
    per-slot combine gate (fused into the PSUM->SBUF copy), scatters them by
    token id into a zeroed [T, D] bf16 buffer (sentinel slots skipped via the
    indirect-DMA bounds check), and one ReduceScatter sums the 8 buffers and
    hands each core its 1024-token shard, which is cast to fp32 as y.
"""

import sys

if "/opt/trn_rl_repo" not in sys.path:
    sys.path.insert(0, "/opt/trn_rl_repo")

import numpy as np
import ml_dtypes

import concourse.bacc as bacc
import concourse.mybir as mybir
import concourse.tile as tile
from concourse import bass
from concourse.bass_utils import run_bass_kernel_spmd

BF16 = mybir.dt.bfloat16
F32 = mybir.dt.float32
I16 = mybir.dt.int16
I32 = mybir.dt.int32
AF = mybir.ActivationFunctionType
OP = mybir.AluOpType

B, S, D, E, F = 4, 2048, 1024, 8, 4096
T = B * S            # 8192 tokens
C = 2 * T // E       # 2048 capacity
NC = 8               # cores
SH = T // NC         # 1024 tokens per shard
CBLK = 512           # FFN slot-block
NCB = C // CBLK      # 4 blocks

LAST_RESULT = None   # BassKernelResults of the most recent run (for profiling)


def _build_program():
    nc = bacc.Bacc("TRN2", target_bir_lowering=False, debug=False, num_devices=NC)

    # ---- per-core external inputs ----
    xT_sh = nc.dram_tensor("xT_sh", [D, SH], F32, kind="ExternalInput").ap()
    xb = nc.dram_tensor("xb", [T + 1, D], BF16, kind="ExternalInput").ap()
    wg_d = nc.dram_tensor("wg", [D, E], F32, kind="ExternalInput").ap()
    wgt_d = nc.dram_tensor("wgt", [D, F], BF16, kind="ExternalInput").ap()
    wdn_d = nc.dram_tensor("wdn", [F, D], BF16, kind="ExternalInput").ap()
    cid_d = nc.dram_tensor("cid", [128, 1], F32, kind="ExternalInput").ap()
    # host-generated constants (gpsimd iota/affine_select aren't available)
    ident_d = nc.dram_tensor("ident", [128, 128], F32, kind="ExternalInput").ap()
    slmat_d = nc.dram_tensor("slmat", [128, 128], F32, kind="ExternalInput").ap()
    tidx_d = nc.dram_tensor("tidx", [128, 64], F32, kind="ExternalInput").ap()
    eidx_d = nc.dram_tensor("eidx", [128, E], F32, kind="ExternalInput").ap()
    y_d = nc.dram_tensor("y", [SH, D], F32, kind="ExternalOutput").ap()

    # ---- internal DRAM ----
    pay_in = nc.dram_tensor("pay_in", [4 * SH], F32).ap()
    pay_all = nc.dram_tensor("pay_all", [NC * 4 * SH], F32, addr_space="Shared").ap()
    rt_d = nc.dram_tensor("rt_d", [T + 1, 4], F32).ap()       # per-token (idx1, g1k, g2k, 0)
    contrib = nc.dram_tensor("contrib", [T, D], BF16).ap()    # this expert's combine contribution
    rs_out = nc.dram_tensor("rs_out", [SH, D], BF16).ap()     # ReduceScatter result (token shard)

    with tile.TileContext(nc) as tc:
        with (
            tc.tile_pool(name="persist", bufs=1) as pp,
            tc.tile_pool(name="psum_s", bufs=2, space="PSUM") as pss,
        ):
            ident = pp.tile([128, 128], F32)
            nc.sync.dma_start(ident[:], ident_d[:])
            ident_bf = pp.tile([128, 128], BF16)
            nc.vector.tensor_copy(ident_bf[:], ident[:])
            cid = pp.tile([128, 1], F32)
            nc.sync.dma_start(cid[:], cid_d[:])
            zeros64 = pp.tile([128, 64], F32)
            nc.vector.memset(zeros64[:], 0.0)
            ones128 = pp.tile([128, 128], F32)
            nc.vector.memset(ones128[:], 1.0)

            # resident expert weights (bf16); DMAs issued after the router
            # section so the router's xT load goes out first
            wgt_sb = pp.tile([128, D // 128, F], BF16)
            wdn_sb = pp.tile([128, F // 128, D], BF16)

            # persistent routing products
            tokc = pp.tile([128, C // 128], I32)    # dispatch: slot->token, col-major
            zbf = pp.tile([128, 1024], BF16)        # zero source for contrib fill

            # =============== ROUTER (token shard, fp32) ===============
            with (
                tc.tile_pool(name="route", bufs=1) as pr,
                tc.tile_pool(name="psum_mg", bufs=1, space="PSUM") as psd,
            ):
                # DMA issue order matters (transfers serialize): router inputs
                # first, then FFN weights, then the contrib zero-fill, which
                # only needs to land before the first FFN scatter
                eidx = pr.tile([128, E], F32)
                nc.sync.dma_start(eidx[:], eidx_d[:])
                sl = pr.tile([128, 128], F32)
                nc.sync.dma_start(sl[:], slmat_d[:])
                tif = pr.tile([128, 64], F32)
                nc.sync.dma_start(tif[:], tidx_d[:])

                xT_a = pr.tile([128, D // 128, SH // 2], F32)
                xT_b = pr.tile([128, D // 128, SH // 2], F32)
                xT_v = xT_sh.rearrange("(o q) t -> q o t", q=128)
                nc.sync.dma_start(xT_a[:], xT_v[:, :, : SH // 2])
                nc.sync.dma_start(xT_b[:], xT_v[:, :, SH // 2 :])
                wg_sb = pr.tile([128, D // 128, E], F32)
                nc.sync.dma_start(wg_sb[:], wg_d.rearrange("(o q) e -> q o e", q=128))

                lg = pr.tile([128, 8, E], F32)  # logits, token pos j = 128*tt + p
                for tt in range(8):
                    half = xT_a if tt < 4 else xT_b
                    toff = 128 * (tt % 4)
                    ps = pss.tile([128, E], F32, space="PSUM", tag="ps_small")
                    for kd in range(8):
                        nc.tensor.matmul(
                            ps[:],
                            lhsT=half[:, kd, toff : toff + 128],
                            rhs=wg_sb[:, kd, :],
                            start=(kd == 0),
                            stop=(kd == 7),
                        )
                    nc.vector.tensor_copy(lg[:, tt, :], ps[:])

                def emax(src, width, tag):
                    red = pr.tile([128, 8, 1], F32, tag=f"emax{tag}")
                    nc.vector.reduce_max(red[:], src[:], axis=mybir.AxisListType.X)
                    return red  # [128, 8, 1]

                m1x = emax(lg, E, "m1")
                is1 = pr.tile([128, 8, E], F32)
                nc.vector.tensor_tensor(out=is1[:], in0=lg[:], in1=m1x[:].to_broadcast([128, 8, E]), op=OP.is_equal)
                l2 = pr.tile([128, 8, E], F32)
                nc.vector.scalar_tensor_tensor(
                    out=l2[:], in0=is1[:], scalar=-1e30, in1=lg[:], op0=OP.mult, op1=OP.add,
                )
                m2x = emax(l2, E, "m2")
                is2 = pr.tile([128, 8, E], F32)
                nc.vector.tensor_tensor(out=is2[:], in0=l2[:], in1=m2x[:].to_broadcast([128, 8, E]), op=OP.is_equal)

                dm = pr.tile([128, 8, 1], F32)
                nc.vector.tensor_tensor(out=dm[:], in0=m2x[:], in1=m1x[:], op=OP.subtract)
                e2 = pr.tile([128, 8, 1], F32)
                nc.scalar.activation(e2[:], dm[:], AF.Exp)
                den = pr.tile([128, 8, 1], F32)
                nc.vector.tensor_scalar_add(den[:], e2[:], 1.0)
                g1 = pr.tile([128, 8, 1], F32)
                nc.vector.reciprocal(g1[:], den[:])
                g2 = pr.tile([128, 8, 1], F32)
                nc.vector.tensor_tensor(out=g2[:], in0=e2[:], in1=g1[:], op=OP.mult)

                def argmax_num(mask, tag):
                    t1 = pr.tile([128, 8, E], F32, tag=f"am_t1{tag}")
                    nc.vector.tensor_tensor(
                        out=t1[:], in0=mask[:], in1=eidx[:, None, :].to_broadcast([128, 8, E]), op=OP.mult,
                    )
                    red = pr.tile([128, 8, 1], F32, tag=f"am_r{tag}")
                    nc.vector.reduce_sum(red[:], t1[:], axis=mybir.AxisListType.X)
                    return red  # [128, 8, 1]

                pk = pr.tile([128, 4, 8], F32)
                nc.vector.tensor_copy(pk[:, 0, :], argmax_num(is1, "a")[:, :, 0])
                nc.vector.tensor_copy(pk[:, 1, :], argmax_num(is2, "b")[:, :, 0])
                nc.vector.tensor_copy(pk[:, 2, :], g1[:, :, 0])
                nc.vector.tensor_copy(pk[:, 3, :], g2[:, :, 0])
                nc.sync.dma_start(pay_in.rearrange("(a p tt) -> p a tt", a=4, p=128), pk[:])

                nc.gpsimd.collective_compute(
                    "AllGather", OP.bypass,
                    replica_groups=[list(range(NC))],
                    ins=[pay_in[:].opt()], outs=[pay_all[:].opt()],
                )

                # reread all 4 arrays into global routing layout [128, 64] (t = 64p + i)
                rt = pr.tile([128, 4, 64], F32)
                pay_view = pay_all.rearrange("(r a p16 i) -> r p16 a i", r=NC, a=4, p16=16)
                for r in range(NC):
                    eng = nc.sync if r % 2 == 0 else nc.scalar
                    eng.dma_start(rt[16 * r : 16 * r + 16, :, :], pay_view[r])
                i1r, i2r = rt[:, 0, :], rt[:, 1, :]
                g1r, g2r = rt[:, 2, :], rt[:, 3, :]

                # gate_proj weights issue on the SP queue BEHIND the payload
                # and the AllGather re-reads (the SP sequencer stalls on the
                # collective there), so this bulk traffic cannot delay the
                # router's critical path; it drains during slot assignment.
                # (down_proj loads + contrib zero-fill are issued inside the
                # FFN loop so their shared-semaphore thresholds cannot stall
                # the first mm1.)
                wgt_v = wgt_d.rearrange("(o q) f -> q o f", q=128)
                for o in range(D // 128):
                    nc.sync.dma_start(wgt_sb[:, o, :], wgt_v[:, o, :])

                # =============== SLOT ASSIGNMENT (replicated) ===============
                # masks for all 8 experts in one broadcast is_equal; per-expert
                # inclusive cumsum via one flat scan + segment-end correction
                m1 = pr.tile([128, E, 64], F32)
                m2 = pr.tile([128, E, 64], F32)
                eb = eidx[:, :, None].to_broadcast([128, E, 64])
                nc.vector.tensor_tensor(out=m1[:], in0=rt[:, 0:1, :].to_broadcast([128, E, 64]), in1=eb, op=OP.is_equal)
                nc.vector.tensor_tensor(out=m2[:], in0=rt[:, 1:2, :].to_broadcast([128, E, 64]), in1=eb, op=OP.is_equal)
                sc1 = pr.tile([128, E, 64], F32)
                sc2 = pr.tile([128, E, 64], F32)
                fl1 = pr.tile([128, E, 64], F32, tag="fl")
                zb = zeros64[:, 0:1].to_broadcast([128, E * 64])
                nc.vector.tensor_tensor_scan(
                    fl1[:].rearrange("p e i -> p (e i)"), m1[:].rearrange("p e i -> p (e i)"),
                    zb, 0.0, op0=OP.add, op1=OP.add,
                )
                nc.vector.tensor_copy(sc1[:, 0, :], fl1[:, 0, :])
                nc.vector.tensor_tensor(
                    out=sc1[:, 1:, :], in0=fl1[:, 1:, :],
                    in1=fl1[:, : E - 1, 63:64].to_broadcast([128, E - 1, 64]), op=OP.subtract,
                )
                fl2 = pr.tile([128, E, 64], F32, tag="fl")
                nc.vector.tensor_tensor_scan(
                    fl2[:].rearrange("p e i -> p (e i)"), m2[:].rearrange("p e i -> p (e i)"),
                    zb, 0.0, op0=OP.add, op1=OP.add,
                )
                nc.vector.tensor_copy(sc2[:, 0, :], fl2[:, 0, :])
                nc.vector.tensor_tensor(
                    out=sc2[:, 1:, :], in0=fl2[:, 1:, :],
                    in1=fl2[:, : E - 1, 63:64].to_broadcast([128, E - 1, 64]), op=OP.subtract,
                )
                tot1 = pr.tile([128, E], F32)
                tot2 = pr.tile([128, E], F32)
                nc.vector.tensor_copy(tot1[:], sc1[:, :, 63])
                nc.vector.tensor_copy(tot2[:], sc2[:, :, 63])

                of1_ps = pss.tile([128, E], F32, space="PSUM", tag="ps_small")
                nc.tensor.matmul(of1_ps[:], lhsT=sl[:], rhs=tot1[:], start=True, stop=True)
                of1 = pr.tile([128, E], F32)
                nc.vector.tensor_scalar_add(of1[:], of1_ps[:], -1.0)
                of2_ps = pss.tile([128, E], F32, space="PSUM", tag="ps_small")
                nc.tensor.matmul(of2_ps[:], lhsT=sl[:], rhs=tot2[:], start=True, stop=False)
                nc.tensor.matmul(of2_ps[:], lhsT=ones128[:], rhs=tot1[:], start=False, stop=True)
                of2 = pr.tile([128, E], F32)
                nc.vector.tensor_scalar_add(of2[:], of2_ps[:], -1.0)

                def loc_s(sc, m, of, tag, eng=None):
                    eng = eng or nc.vector
                    tmp = pr.tile([128, E, 64], F32, tag=f"loc_tmp{tag}")
                    eng.tensor_tensor(
                        out=tmp[:], in0=sc[:],
                        in1=of[:, :, None].to_broadcast([128, E, 64]), op=OP.add,
                    )
                    eng.tensor_tensor(out=tmp[:], in0=tmp[:], in1=m[:], op=OP.mult)
                    red = pr.tile([128, 64, 1], F32, tag=f"loc_r{tag}")
                    nc.vector.reduce_sum(
                        red[:], tmp[:].rearrange("p e i -> p i e"), axis=mybir.AxisListType.X,
                    )
                    return red  # [128, 64, 1]

                l1s = loc_s(sc1, m1, of1, "a")[:, :, 0]
                l2s = loc_s(sc2, m2, of2, "b")[:, :, 0]

                def keep_g(ls, gr, tag, eng=None):
                    eng = eng or nc.vector
                    kp = pr.tile([128, 64], F32, tag=f"kp{tag}")
                    eng.tensor_scalar(out=kp[:], in0=ls, scalar1=float(C), scalar2=None, op0=OP.is_lt)
                    gk = pr.tile([128, 64], F32, tag=f"gk{tag}")
                    eng.tensor_tensor(out=gk[:], in0=gr, in1=kp[:], op=OP.mult)
                    return gk, kp

                g1k, kp1 = keep_g(l1s, g1r, "a")
                g2k, kp2 = keep_g(l2s, g2r, "b")

                # per-token routing table -> DRAM: row t = (idx1, g1k, g2k, 0)
                rtt = pr.tile([128, 64, 4], F32)
                nc.vector.tensor_copy(rtt[:, :, 0], i1r)
                nc.vector.tensor_copy(rtt[:, :, 1], g1k[:])
                nc.vector.tensor_copy(rtt[:, :, 2], g2k[:])
                nc.vector.memset(rtt[:, :, 3], 0.0)
                nc.sync.dma_start(rt_d[0:T, :].rearrange("(p i) c -> p i c", p=128), rtt[:])
                nc.sync.dma_start(rt_d[T : T + 1, :], zeros64[0:1, 0:4])

                # ====== SLOT -> TOKEN MAP (local_scatter + merge + diagonal) ======
                tp1 = pr.tile([128, 64], F32)
                nc.vector.tensor_scalar_add(tp1[:], tif[:], 1.0)   # token id + 1

                def slot_halves(ls, ir, kp, tag, eng=None):
                    eng = eng or nc.vector
                    # sel = (expert == cid) && kept; slot+1 where selected else 0
                    isc = pr.tile([128, 64], F32, tag=f"isc{tag}")
                    eng.tensor_tensor(out=isc[:], in0=ir, in1=cid[:, 0:1].to_broadcast([128, 64]), op=OP.is_equal)
                    sel = pr.tile([128, 64], F32, tag=f"sel{tag}")
                    eng.tensor_tensor(out=sel[:], in0=isc[:], in1=kp[:], op=OP.mult)
                    sp1 = pr.tile([128, 64], F32, tag=f"sp1{tag}")  # sel ? slot+1 : 0
                    eng.tensor_scalar_add(sp1[:], ls, 1.0)
                    eng.tensor_tensor(out=sp1[:], in0=sp1[:], in1=sel[:], op=OP.mult)
                    # lo half: slot in [0, 1024): idx = slot, else -1
                    mlo = pr.tile([128, 64], F32, tag=f"mlo{tag}")
                    eng.tensor_scalar(out=mlo[:], in0=sp1[:], scalar1=1024.0, scalar2=None, op0=OP.is_le)
                    eng.tensor_tensor(out=mlo[:], in0=mlo[:], in1=sel[:], op=OP.mult)
                    ilo = pr.tile([128, 64], F32, tag=f"ilo{tag}")
                    eng.tensor_tensor(out=ilo[:], in0=mlo[:], in1=sp1[:], op=OP.mult)
                    eng.tensor_scalar_add(ilo[:], ilo[:], -1.0)
                    # hi half: slot in [1024, 2048): idx = slot - 1024, else -1
                    mhi = pr.tile([128, 64], F32, tag=f"mhi{tag}")
                    eng.tensor_scalar(out=mhi[:], in0=sp1[:], scalar1=1024.0, scalar2=None, op0=OP.is_gt)
                    ihi = pr.tile([128, 64], F32, tag=f"ihi{tag}")
                    eng.tensor_scalar_add(ihi[:], sp1[:], -1024.0)
                    eng.tensor_tensor(out=ihi[:], in0=ihi[:], in1=mhi[:], op=OP.mult)
                    eng.tensor_scalar_add(ihi[:], ihi[:], -1.0)
                    return ilo, ihi

                i1lo, i1hi = slot_halves(l1s, i1r, kp1, "a")
                i2lo, i2hi = slot_halves(l2s, i2r, kp2, "b")

                data128 = pr.tile([128, 128], I16)
                nc.vector.tensor_copy(data128[:, :64], tp1[:])
                nc.vector.tensor_copy(data128[:, 64:], tp1[:])
                idxlo = pr.tile([128, 128], I16)
                nc.vector.tensor_copy(idxlo[:, :64], i1lo[:])
                nc.vector.tensor_copy(idxlo[:, 64:], i2lo[:])
                idxhi = pr.tile([128, 128], I16)
                nc.vector.tensor_copy(idxhi[:, :64], i1hi[:])
                nc.vector.tensor_copy(idxhi[:, 64:], i2hi[:])

                dst_lo = pr.tile([128, 1024], I16)
                nc.gpsimd.local_scatter(dst_lo[:], data128[:], idxlo[:], channels=128, num_elems=1024, num_idxs=128)
                dst_hi = pr.tile([128, 1024], I16)
                nc.gpsimd.local_scatter(dst_hi[:], data128[:], idxhi[:], channels=128, num_elems=1024, num_idxs=128)

                # merge across partitions with a ones-matmul, keep results in
                # PSUM (4 banks) and run the diagonal extraction straight off
                # them: tokraw[p, k] = merged-flat[128k + p]
                tokraw = pr.tile([128, C // 128], F32)
                scratch = pr.tile([128, 128], F32, tag="diag_scr")
                for half, dst in ((0, dst_lo), (1, dst_hi)):
                    dstf = pr.tile([128, 1024], F32, tag="dstf")
                    nc.vector.tensor_copy(dstf[:], dst[:])
                    for ch in range(2):
                        mg_ps = psd.tile([128, 512], F32, space="PSUM", tag=f"ps_mg{2 * half + ch}")
                        nc.tensor.matmul(mg_ps[:], lhsT=ones128[:], rhs=dstf[:, 512 * ch : 512 * (ch + 1)], start=True, stop=True)
                        for kk in range(4):
                            k = 4 * (2 * half + ch) + kk
                            nc.vector.scalar_tensor_tensor(
                                out=scratch[:], in0=mg_ps[:, 128 * kk : 128 * (kk + 1)], scalar=0.0,
                                in1=ident[:], op0=OP.add, op1=OP.mult,
                                accum_out=tokraw[:, k : k + 1],
                            )
                # sanitize: 0 -> T (zero row); v -> v-1
                iszero = pr.tile([128, C // 128], F32)
                nc.vector.tensor_scalar(out=iszero[:], in0=tokraw[:], scalar1=0.0, scalar2=None, op0=OP.is_equal)
                nc.vector.scalar_tensor_tensor(
                    out=tokraw[:], in0=iszero[:], scalar=float(T + 1), in1=tokraw[:], op0=OP.mult, op1=OP.add,
                )
                nc.vector.tensor_scalar_add(tokraw[:], tokraw[:], -1.0)
                nc.vector.tensor_copy(tokc[:], tokraw[:])

            # =============== EXPERT FFN (bf16) ===============
            with (
                tc.tile_pool(name="ffn", bufs=1) as pf,
                tc.tile_pool(name="ffn_db", bufs=2) as pfd,
                tc.tile_pool(name="psum_mm", bufs=2, space="PSUM") as psm,
            ):
                def fetch_block(cb):
                    # token rows + routing rows for block cb (prefetched one
                    # block ahead so the Pool-queue DMAs overlap compute)
                    drowb = pfd.tile([128, CBLK // 128, D], BF16, tag="drowb")
                    rtg = pfd.tile([128, CBLK // 128, 4], F32, tag="rtg")
                    for kt in range(CBLK // 128):
                        k = (CBLK // 128) * cb + kt
                        nc.gpsimd.indirect_dma_start(
                            out=drowb[:, kt, :], out_offset=None, in_=xb[:],
                            in_offset=bass.IndirectOffsetOnAxis(ap=tokc[:, k : k + 1], axis=0),
                        )
                        nc.gpsimd.indirect_dma_start(
                            out=rtg[:, kt, :], out_offset=None, in_=rt_d[:],
                            in_offset=bass.IndirectOffsetOnAxis(ap=tokc[:, k : k + 1], axis=0),
                        )
                    return drowb, rtg

                def transpose_block(drowb):
                    dispT = pfd.tile([128, D // 128, CBLK], BF16, tag="dispT")
                    for kt in range(CBLK // 128):
                        for dt in range(D // 128):
                            tr_ps = psm.tile([128, 128], BF16, space="PSUM", tag="ps_tr")
                            nc.tensor.transpose(tr_ps[:], drowb[:, kt, 128 * dt : 128 * (dt + 1)], ident_bf[:])
                            nc.vector.tensor_copy(dispT[:, dt, 128 * kt : 128 * (kt + 1)], tr_ps[:])
                    return dispT

                nxt = fetch_block(0)
                dispT = transpose_block(nxt[0])
                for cb in range(NCB):
                    drowb, rtg = nxt
                    if cb + 1 < NCB:
                        nxt = fetch_block(cb + 1)
                    # combine gate for this block's slots: idx1 match -> g1 else g2
                    gate_b = pfd.tile([128, CBLK // 128], F32, tag="gate_b")
                    isc = pfd.tile([128, CBLK // 128], F32, tag="isc")
                    nc.vector.tensor_tensor(
                        out=isc[:], in0=rtg[:, :, 0],
                        in1=cid[:, 0:1].to_broadcast([128, CBLK // 128]), op=OP.is_equal,
                    )
                    nc.vector.tensor_tensor(out=gate_b[:], in0=rtg[:, :, 1], in1=rtg[:, :, 2], op=OP.subtract)
                    nc.vector.tensor_tensor(out=gate_b[:], in0=gate_b[:], in1=isc[:], op=OP.mult)
                    nc.vector.tensor_tensor(out=gate_b[:], in0=gate_b[:], in1=rtg[:, :, 2], op=OP.add)

                    hT = pf.tile([128, F // 128, CBLK], BF16, tag="hT")
                    for ft in range(F // 128):
                        ps1 = psm.tile([128, CBLK], F32, space="PSUM", tag="ps1")
                        for kd in range(D // 128):
                            nc.tensor.matmul(
                                ps1[:],
                                lhsT=wgt_sb[:, kd, 128 * ft : 128 * ft + 128],
                                rhs=dispT[:, kd, :],
                                start=(kd == 0), stop=(kd == D // 128 - 1),
                            )
                        nc.scalar.activation(hT[:, ft, :], ps1[:], AF.Gelu)

                    if cb == 0:
                        # down_proj weights: linearly after mm1 so their DMAs
                        # never inflate mm1's wait thresholds; requested right
                        # behind the gate_proj chunks on the SP queue
                        wdn_v = wdn_d.rearrange("(o q) d -> q o d", q=128)
                        for o in range(F // 128 // 4):
                            nc.sync.dma_start(wdn_sb[:, 4 * o : 4 * o + 4, :], wdn_v[:, 4 * o : 4 * o + 4, :])
                        # contrib zero-fill (Activation queue), gated on the
                        # last wdn chunk so its traffic strictly follows the
                        # weights; linearly before the first scatter, which
                        # genuinely depends on it
                        nc.vector.tensor_scalar(
                            out=zbf[:], in0=wdn_sb[:, F // 128 - 1, :],
                            scalar1=0.0, scalar2=None, op0=OP.mult,
                        )
                        zc = contrib.rearrange("(o p) d -> p o d", p=128)
                        for q in range(T // 128):
                            nc.scalar.dma_start(zc[:, q, :], zbf[:])
                    # transposes for the next block run on the PE here, so
                    # dispT is ready the moment this block's mm2 retires
                    if cb + 1 < NCB:
                        dispT_next = transpose_block(nxt[0])

                    # mm2 with swapped operands: eo[c, d] = hT.T @ w_down -> row-major
                    # out, gated by the per-slot combine weight in the PSUM copy
                    eo_sb = pf.tile([128, CBLK // 128, D], BF16, tag="eo_sb")
                    for ct in range(CBLK // 128):
                        k = (CBLK // 128) * cb + ct
                        for dc in range(D // 512):
                            ps2 = psm.tile([128, 512], F32, space="PSUM", tag="ps2")
                            for ft in range(F // 128):
                                nc.tensor.matmul(
                                    ps2[:],
                                    lhsT=hT[:, ft, 128 * ct : 128 * ct + 128],
                                    rhs=wdn_sb[:, ft, 512 * dc : 512 * dc + 512],
                                    start=(ft == 0), stop=(ft == F // 128 - 1),
                                )
                            nc.vector.tensor_scalar_mul(
                                eo_sb[:, ct, 512 * dc : 512 * dc + 512], ps2[:],
                                gate_b[:, ct : ct + 1],
                            )
                        # scatter gated rows to their token position; sentinel
                        # (empty-slot) indices == T fail the bounds check and
                        # are silently dropped. The out AP is a 128-row window:
                        # indirect addressing only uses its base + row stride,
                        # and the sliced AP sizes the DMA as the 256 KiB it
                        # actually moves rather than the whole [T, D] tensor.
                        nc.gpsimd.indirect_dma_start(
                            out=contrib[0:128, :], out_offset=bass.IndirectOffsetOnAxis(ap=tokc[:, k : k + 1], axis=0),
                            in_=eo_sb[:, ct, :], in_offset=None,
                            bounds_check=T - 1, oob_is_err=False,
                        )
                    if cb + 1 < NCB:
                        dispT = dispT_next

                nc.gpsimd.collective_compute(
                    "ReduceScatter", OP.add,
                    replica_groups=[list(range(NC))],
                    ins=[contrib[:].opt()], outs=[rs_out[:].opt()],
                )

            # =============== OUTPUT CAST (token shard) ===============
            with tc.tile_pool(name="tail", bufs=2) as pt:
                NH = 2
                for h in range(NH):
                    nch = SH // 128 // NH
                    rsb = pt.tile([128, nch, D], BF16, tag="rsb")
                    nc.sync.dma_start(
                        rsb[:],
                        rs_out[SH // NH * h : SH // NH * (h + 1), :].rearrange("(c p) d -> p c d", p=128),
                    )
                    acc = pt.tile([128, nch, D], F32, tag="acc")
                    nc.vector.tensor_copy(acc[:], rsb[:])
                    nc.sync.dma_start(
                        y_d[SH // NH * h : SH // NH * (h + 1), :].rearrange("(c p) d -> p c d", p=128),
                        acc[:],
                    )

    nc.compile()
    return nc


_PROGRAM = None


def _get_program():
    global _PROGRAM
    if _PROGRAM is None:
        _PROGRAM = _build_program()
    return _PROGRAM


def host_constants():
    p = np.arange(128)
    return {
        "ident": np.eye(128, dtype=np.float32),
        "slmat": (np.arange(128)[None, :] > p[:, None]).astype(np.float32),
        "tidx": (64 * p[:, None] + np.arange(64)[None, :]).astype(np.float32),
        "eidx": np.tile(np.arange(E, dtype=np.float32), (128, 1)),
    }


def kernel(x, wg, w_gate, w_down, _trace=False):
    global LAST_RESULT
    x = np.asarray(x, np.float32)
    wg_np = np.asarray(wg, np.float32)
    w_gate_np = np.asarray(w_gate, np.float32)
    w_down_np = np.asarray(w_down, np.float32)

    tokens = x.reshape(T, D)
    xb = np.zeros((T + 1, D), ml_dtypes.bfloat16)
    xb[:T] = tokens.astype(ml_dtypes.bfloat16)

    # shard m holds tokens [SH*m, SH*(m+1)); its xT columns are permuted so that
    # matmul tile position j = 128*tt + p corresponds to local token 8*p + tt,
    # making the routing payload DMA contiguous.
    j = np.arange(SH)
    perm = 8 * (j % 128) + j // 128  # local token index at column position j
    consts = host_constants()

    in_maps = []
    for m in range(NC):
        shard = tokens[SH * m : SH * (m + 1)]
        xT_sh = np.ascontiguousarray(shard[perm].T)
        in_maps.append({
            "xT_sh": xT_sh,
            "xb": xb,
            "wg": wg_np,
            "wgt": np.ascontiguousarray(w_gate_np[m].astype(ml_dtypes.bfloat16)),
            "wdn": np.ascontiguousarray(w_down_np[m].astype(ml_dtypes.bfloat16)),
            "cid": np.full((128, 1), float(m), np.float32),
            **consts,
        })

    nc = _get_program()
    res = run_bass_kernel_spmd(nc, in_maps, core_ids=list(range(NC)), trace=_trace)
    LAST_RESULT = res
    out = np.concatenate([res.results[m]["y"] for m in range(NC)], axis=0)
    return out.reshape(B, S, D).astype(x.dtype)


def bench(x, wg, w_gate, w_down, iters=6):
    """Measure per-execution wall time with device-resident inputs.

    Returns (output, per_call_seconds_list, overhead_seconds_list) where
    overhead is measured with an empty jitted identity on the same mesh.
    """
    import time
    import jax
    from jax.sharding import Mesh, PartitionSpec, NamedSharding
    from jax.experimental.shard_map import shard_map
    import concourse.mybir as _mybir
    from concourse.bass2jax import _bass_exec_p, install_neuronx_cc_hook, partition_id_tensor

    install_neuronx_cc_hook()
    nc = _get_program()

    x = np.asarray(x, np.float32)
    tokens = x.reshape(T, D)
    xb = np.zeros((T + 1, D), ml_dtypes.bfloat16)
    xb[:T] = tokens.astype(ml_dtypes.bfloat16)
    j = np.arange(SH)
    perm = 8 * (j % 128) + j // 128
    consts = host_constants()
    w_gate_np = np.asarray(w_gate, np.float32)
    w_down_np = np.asarray(w_down, np.float32)
    in_maps = []
    for m in range(NC):
        shard = tokens[SH * m : SH * (m + 1)]
        in_maps.append({
            "xT_sh": np.ascontiguousarray(shard[perm].T),
            "xb": xb,
            "wg": np.asarray(wg, np.float32),
            "wgt": np.ascontiguousarray(w_gate_np[m].astype(ml_dtypes.bfloat16)),
            "wdn": np.ascontiguousarray(w_down_np[m].astype(ml_dtypes.bfloat16)),
            "cid": np.full((128, 1), float(m), np.float32),
            **consts,
        })

    in_names, out_names, out_avals, zero_outs = [], [], [], []
    for alloc in nc.m.functions[0].allocations:
        if not isinstance(alloc, _mybir.MemoryLocationSet):
            continue
        name = alloc.memorylocations[0].name
        if alloc.kind == "ExternalInput":
            if nc.partition_id_tensor is None or name != nc.partition_id_tensor.name:
                in_names.append(name)
        elif alloc.kind == "ExternalOutput":
            shape = tuple(alloc.tensor_shape)
            dtype = _mybir.dt.np(alloc.dtype)
            out_names.append(name)
            out_avals.append(jax.core.ShapedArray(shape, dtype))
            zero_outs.append(np.zeros(shape, dtype))
    n_params = len(in_names)
    all_in_names = in_names + out_names
    if nc.partition_id_tensor is not None:
        all_in_names = all_in_names + [nc.partition_id_tensor.name]

    def _body(*args):
        operands = list(args)
        if nc.partition_id_tensor is not None:
            operands.append(partition_id_tensor())
        outs = _bass_exec_p.bind(
            *operands,
            out_avals=tuple(out_avals),
            in_names=tuple(all_in_names),
            out_names=tuple(out_names),
            lowering_input_output_aliases=(),
            sim_require_finite=True,
            sim_require_nnan=True,
            nc=nc,
        )
        return tuple(outs)

    devices = jax.devices()[:NC]
    mesh = Mesh(np.asarray(devices), ("core",))
    nsh = NamedSharding(mesh, PartitionSpec("core"))
    n_outs = len(out_avals)
    donate = tuple(range(n_params, n_params + n_outs))
    sharded = jax.jit(
        shard_map(_body, mesh=mesh, in_specs=(PartitionSpec("core"),) * (n_params + n_outs),
                  out_specs=(PartitionSpec("core"),) * n_outs, check_rep=False),
        donate_argnums=donate, keep_unused=True,
    )

    concat_in = [
        jax.device_put(np.concatenate([np.asarray(in_maps[c][nm]) for c in range(NC)], axis=0), nsh)
        for nm in in_names
    ]
    zero_sets = [
        [jax.device_put(np.zeros((NC * z.shape[0], *z.shape[1:]), z.dtype), nsh) for z in zero_outs]
        for _ in range(iters + 1)
    ]

    out = sharded(*concat_in, *zero_sets[0])  # warmup + compile
    jax.block_until_ready(out)
    times = []
    for it in range(iters):
        t0 = time.perf_counter()
        out = sharded(*concat_in, *zero_sets[it + 1])
        jax.block_until_ready(out)
        times.append(time.perf_counter() - t0)

    outs = {
        nm: np.asarray(out[i]).reshape(NC, *out_avals[i].shape) for i, nm in enumerate(out_names)
    }
    y = np.concatenate([outs["y"][m] for m in range(NC)], axis=0).reshape(B, S, D).astype(x.dtype)
    return y, times

